# revision 24
# baseline (speedup 1.0000x reference)
"""Trainium2 Bass kernel for GNN mean-aggregation message passing.

  m = relu(concat(y[src], ex) @ W1.T + b1)        per edge
  z = segment_mean(m, dst)                        per node (0 for isolated)
  h = relu(z @ W2.T + b2)                         per node

Strategy (8 NeuronCores, one SPMD program, dst-sharded):
  - Host shards edges by dst node range (N/8 nodes per core) and sorts each
    core's edges by (psum-window, dst). Per-window tile counts and per-tile
    one-hot column spans are unioned across cores so a single program fits
    all shards (padding slots carry zero features and zero one-hot weight).
  - The full per-edge feature matrix feat = [y[src]; ex; 1] is gathered on
    the HOST into a pair-packed edge-slot layout ([98, E_slots/2] bf16 per
    core: tile 2i in rows 0:49, tile 2i+1 in rows 49:98), so the device
    never does an indirect gather, and one PE matmul against the
    block-diagonal [[W1';0],[0;W1']] computes TWO 128-edge message tiles
    per LDWEIGHTS (bias folded via the ones row) -> relu -> m.
  - Scatter-sum on PE: s^T[48, win] += m[128e, 48].T @ O[128e, span], with
    O a 0/1 one-hot (exact in fp8). The mean is applied at window drain
    as an elementwise multiply with 1/deg -> bf16 z.
  - Node MLP: h^T[32, win] = relu(W2.T @ z^T + b2) streamed per window.
"""

import numpy as np
import ml_dtypes

N_CORES = 8
WIN = 512          # nodes per PSUM scatter window (1 bank, no span splits)
TILE_E = 128       # edges per scatter matmul (PE contraction dim)
SUPER = 8          # tiles (= SUPER//2 pairs) per relu batch

BF16 = ml_dtypes.bfloat16


def _preprocess(y, ex, W1, b1, W2, b2, src, dst):
    N, ND = y.shape
    E, ED = ex.shape
    D = ND + ED
    K1 = D + 1
    NPC = N // N_CORES
    NW = (NPC + WIN - 1) // WIN

    cnt = np.bincount(dst, minlength=N)
    inv_cnt = (1.0 / np.maximum(cnt, 1)).astype(np.float32)

    core_of = (dst // NPC).astype(np.int64)
    win_of = ((dst - core_of * NPC) // WIN).astype(np.int64)
    cw = core_of * NW + win_of
    key = cw * np.int64(N + 1) + dst
    order = np.argsort(key, kind="stable")

    dst_s = dst[order].astype(np.int64)
    ex_s = ex[order]
    src_s = src[order].astype(np.int64)
    ys = y[src_s]
    core_s = core_of[order]
    win_s = win_of[order]
    cw_s = cw[order]

    cw_cnt = np.bincount(cw_s, minlength=N_CORES * NW).reshape(N_CORES, NW)
    T_w = (cw_cnt.max(axis=0) + TILE_E - 1) // TILE_E          # [NW]
    T_w = ((T_w + 1) // 2) * 2                                 # even (pairs)
    win_block_base = np.concatenate([[0], np.cumsum(T_w)])
    B_tot = int(win_block_base[-1])
    E_slots = B_tot * TILE_E

    # rank of each edge within its (core, window) run
    cw_start = np.zeros(N_CORES * NW + 1, np.int64)
    cw_start[1:] = np.cumsum(cw_cnt.reshape(-1))
    rank = np.arange(E, dtype=np.int64) - cw_start[cw_s]
    slot = win_block_base[win_s] * TILE_E + rank
    tile_of = slot // TILE_E
    p_in_tile = slot % TILE_E

    # per-tile node span (relative to window start), unioned over cores
    rel = dst_s - core_s * NPC - win_s * WIN
    lo_t = np.full(B_tot, np.int64(1 << 60))
    hi_t = np.full(B_tot, np.int64(-1))
    np.minimum.at(lo_t, tile_of, rel)
    np.maximum.at(hi_t, tile_of, rel)
    empty = hi_t < 0
    lo_t[empty] = 0
    hi_t[empty] = 0
    span_t = hi_t - lo_t + 1
    col_off = np.concatenate([[0], np.cumsum(span_t)])
    C_tot = int(col_off[-1])
    o_col = col_off[tile_of] + (rel - lo_t[tile_of])

    # pair-packed feature layout: tile 2i -> rows 0:K1, tile 2i+1 -> K1:2*K1
    fcol = (tile_of // 2) * TILE_E + p_in_tile
    frow = (tile_of % 2) * K1
    featT = np.zeros((N_CORES, 2 * K1, E_slots // 2), BF16)
    O_a = np.zeros((N_CORES, TILE_E, C_tot), ml_dtypes.float8_e4m3)
    for c in range(N_CORES):
        m = core_s == c
        fc = fcol[m]
        fr = frow[m]
        ysb = ys[m].astype(BF16)
        exb = ex_s[m].astype(BF16)
        for half in range(2):
            hm = fr == half * K1
            base = half * K1
            featT[c, base : base + ND, fc[hm]] = ysb[hm]
            featT[c, base + ND : base + D, fc[hm]] = exb[hm]
            featT[c, base + D, fc[hm]] = 1.0
        O_a[c, p_in_tile[m], o_col[m]] = 1.0

    cinv = np.empty((N_CORES, D, NPC), BF16)
    for c in range(N_CORES):
        cinv[c] = np.broadcast_to(inv_cnt[c * NPC : (c + 1) * NPC], (D, NPC))

    meta = {
        "N": N, "E": E, "ND": ND, "ED": ED, "D": D, "NPC": NPC,
        "n_win": NW, "T_w": T_w, "win_block_base": win_block_base,
        "B_tot": B_tot, "E_slots": E_slots,
        "C_tot": C_tot, "lo_t": lo_t, "span_t": span_t, "col_off": col_off,
    }
    W1f = np.concatenate([W1.T, b1[None, :]], 0).astype(BF16)   # [K1, D]
    W1f2 = np.zeros((2 * K1, 2 * D), BF16)
    W1f2[:K1, :D] = W1f
    W1f2[K1:, D:] = W1f
    consts = dict(
        W1f2=np.ascontiguousarray(W1f2),
        W2b=np.ascontiguousarray(W2.T).astype(BF16),
        b2=np.ascontiguousarray(b2.reshape(-1, 1)).astype(np.float32),
    )
    per_core = dict(featT=featT, O=O_a, cinv=cinv)
    return consts, per_core, meta


def _split_excess_waits(nc, mybir):
    """This walrus build accepts at most 1 sync wait per instruction (0 on
    Drain). Move extras onto NOPs inserted just before, same engine."""
    for fn in nc.m.functions:
        for bb in fn.blocks:
            new_list = []
            for ins in bb.instructions:
                si = ins.sync_info
                limit = 0 if isinstance(ins, mybir.InstDrain) else 1
                if si is not None and si.on_wait and len(si.on_wait) > limit:
                    waits = list(si.on_wait)
                    keep, extra = waits[:limit], waits[limit:]
                    while extra:
                        chunk, extra = extra[:1], extra[1:]
                        nop = mybir.InstNoOp(
                            name=nc.get_next_instruction_name(), ins=[], outs=[])
                        nop.engine = ins.engine
                        nop.sync_info = mybir.SyncInfo(on_wait=chunk, on_update=[])
                        nc.register_instruction(nop)
                        new_list.append(nop)
                    si.on_wait = keep
                new_list.append(ins)
            bb.instructions[:] = new_list


def _build_program(meta):
    import concourse.bacc as bacc
    import concourse.mybir as mybir
    import concourse.tile as tile

    f32 = mybir.dt.float32
    bf16 = mybir.dt.bfloat16
    f8 = mybir.dt.float8e4
    Relu = mybir.ActivationFunctionType.Relu
    MULT = mybir.AluOpType.mult
    MAX = mybir.AluOpType.max

    N, ND, ED, D = meta["N"], meta["ND"], meta["ED"], meta["D"]
    K1 = D + 1
    NPC, NW = meta["NPC"], meta["n_win"]
    T_w, wbb = meta["T_w"], meta["win_block_base"]
    E_slots, C_tot = meta["E_slots"], meta["C_tot"]
    lo_t, span_t, col_off = meta["lo_t"], meta["span_t"], meta["col_off"]
    OD = 32

    nc = bacc.Bacc("TRN2")
    feat_ext = nc.dram_tensor("featT", [2 * K1, E_slots // 2], bf16,
                              kind="ExternalInput")
    O_ext = nc.dram_tensor("Omat", [TILE_E, C_tot], f8, kind="ExternalInput")
    cinv_ext = nc.dram_tensor("cinv", [D, NPC], bf16, kind="ExternalInput")
    w1f2_ext = nc.dram_tensor("W1f2", [2 * K1, 2 * D], bf16,
                              kind="ExternalInput")
    w2b_ext = nc.dram_tensor("W2b", [D, OD], bf16, kind="ExternalInput")
    b2_ext = nc.dram_tensor("b2", [OD, 1], f32, kind="ExternalInput")
    out_ext = nc.dram_tensor("hT", [OD, NPC], f32, kind="ExternalOutput")

    with tile.TileContext(nc) as tc:
        with (
            tc.tile_pool(name="const", bufs=1) as cpool,
            tc.tile_pool(name="io", bufs=3) as iopool,
            tc.tile_pool(name="aux", bufs=3) as apool,
            tc.tile_pool(name="msb", bufs=4) as mpool,
            tc.tile_pool(name="psA", bufs=3, space="PSUM") as psA,
            tc.tile_pool(name="psZ", bufs=3, space="PSUM") as psZ,
            tc.tile_pool(name="psH", bufs=2, space="PSUM") as psH,
        ):
            w1f2_sb = cpool.tile([2 * K1, 2 * D], bf16)
            nc.sync.dma_start(out=w1f2_sb[:], in_=w1f2_ext[:])
            w2b_sb = cpool.tile([D, OD], bf16)
            nc.sync.dma_start(out=w2b_sb[:], in_=w2b_ext[:])
            b2_sb = cpool.tile([OD, 1], f32)
            nc.sync.dma_start(out=b2_sb[:], in_=b2_ext[:])
            zl_bf = cpool.tile([1, D], bf16)
            nc.any.memset(zl_bf[:], 0)
            zr_bf = cpool.tile([1, 512], bf16)
            nc.any.memset(zr_bf[:], 0)

            # Software-pipelined emission: scatters trail their msg supers by
            # TWO supers so the ACT/DVE relu (split in half across both
            # engines) has a full super period of slack before the scatter
            # needs m_sb. Window drains trail by a further two supers.
            from collections import deque
            state = {"scatters": deque(), "drain_due": None, "drain_next": None}

            def pop_pending():
                if state["drain_due"] is not None:
                    state["drain_due"]()
                state["drain_due"] = state["drain_next"]
                state["drain_next"] = None
                if len(state["scatters"]) > 2:
                    state["scatters"].popleft()()

            def flush_pending():
                while (state["scatters"] or state["drain_due"] is not None
                       or state["drain_next"] is not None):
                    if state["drain_due"] is not None:
                        state["drain_due"]()
                    state["drain_due"] = state["drain_next"]
                    state["drain_next"] = None
                    if state["scatters"]:
                        state["scatters"].popleft()()

            for w in range(NW):
                B = int(T_w[w])
                if B == 0:
                    continue
                wn = min(WIN, NPC - w * WIN)
                b0 = int(wbb[w])
                gn2 = (B // 2) * TILE_E          # pair-packed columns
                e20 = (b0 // 2) * TILE_E

                feat_t = iopool.tile([2 * K1, gn2], bf16, tag="feat")
                nc.gpsimd.dma_start(
                    out=feat_t[:], in_=feat_ext[:, e20 : e20 + gn2])
                csp = int(col_off[b0 + B] - col_off[b0])
                o_t = apool.tile([TILE_E, csp], f8, tag="omat")
                nc.scalar.dma_start(
                    out=o_t[:],
                    in_=O_ext[:, int(col_off[b0]) : int(col_off[b0]) + csp])
                cinv_t = apool.tile([D, WIN], bf16, tag="cinv")
                nc.sync.dma_start(
                    out=cinv_t[:, :wn], in_=cinv_ext[:, w * WIN : w * WIN + wn])

                psz = psZ.tile([D, WIN], f32, tag="psz")
                nc.tensor.matmul(
                    psz[:, :WIN], zl_bf[:], zr_bf[:, :WIN],
                    start=True, stop=True)

                n_super = (B + SUPER - 1) // SUPER
                for si in range(n_super):
                    s = si * SUPER
                    sb = min(SUPER, B - s)
                    npair = (sb + 1) // 2
                    ps_a = psA.tile([TILE_E, (SUPER // 2) * 2 * D], f32,
                                    tag="psa")
                    for pr in range(npair):
                        p0 = (s // 2) + pr
                        nc.tensor.matmul(
                            ps_a[:, pr * 2 * D : (pr + 1) * 2 * D],
                            feat_t[:, p0 * TILE_E : (p0 + 1) * TILE_E],
                            w1f2_sb[:], start=True, stop=True)
                    m_sb = mpool.tile([TILE_E, SUPER * D], bf16, tag="m")
                    # relu emitted immediately, split ACT/DVE so m_sb is
                    # ready in half the latency
                    half = (sb * D) // 2
                    nc.scalar.activation(
                        out=m_sb[:, :half], in_=ps_a[:, :half], func=Relu)
                    nc.vector.tensor_scalar(
                        out=m_sb[:, half : sb * D], in0=ps_a[:, half : sb * D],
                        scalar1=0.0, scalar2=None, op0=MAX)
                    pop_pending()

                    def _scatter(w=w, wn=wn, b0=b0, s=s, sb=sb, si=si,
                                 n_super=n_super, m_sb=m_sb, psz=psz,
                                 o_t=o_t, cinv_t=cinv_t):
                        for t in range(sb):
                            bt = b0 + s + t
                            lo = int(lo_t[bt])
                            sp = int(span_t[bt])
                            off = int(col_off[bt] - col_off[b0])
                            last = si == n_super - 1 and t == sb - 1
                            nc.tensor.matmul(
                                psz[:, lo : lo + sp],
                                m_sb[:, t * D : (t + 1) * D],
                                o_t[:, off : off + sp],
                                start=False, stop=last,
                                skip_group_check=True)
                        if last:
                            zt = mpool.tile([D, WIN], bf16, tag="zt")
                            nc.vector.tensor_tensor(
                                out=zt[:, :wn], in0=psz[:, :wn],
                                in1=cinv_t[:, :wn], op=MULT)

                            def _drain(w=w, wn=wn, zt=zt):
                                ps_h = psH.tile([OD, WIN], f32, tag="psh")
                                nc.tensor.matmul(
                                    ps_h[:, :wn], w2b_sb[:], zt[:, :wn],
                                    start=True, stop=True)
                                h_sb = mpool.tile([OD, WIN], f32, tag="h")
                                nc.scalar.activation(
                                    out=h_sb[:, :wn], in_=ps_h[:, :wn],
                                    func=Relu, bias=b2_sb[:, 0:1])
                                nc.sync.dma_start(
                                    out=out_ext[:, w * WIN : w * WIN + wn],
                                    in_=h_sb[:, :wn])

                            state["drain_next"] = _drain

                    state["scatters"].append(_scatter)
            flush_pending()

    nc.compile()
    _split_excess_waits(nc, mybir)
    return nc


def kernel(y, ex, W1, b1, W2, b2, src, dst):
    from concourse.bass_utils import run_bass_kernel_spmd

    y = np.asarray(y, dtype=np.float32)
    ex = np.asarray(ex, dtype=np.float32)
    W1 = np.asarray(W1, dtype=np.float32)
    b1 = np.asarray(b1, dtype=np.float32)
    W2 = np.asarray(W2, dtype=np.float32)
    b2 = np.asarray(b2, dtype=np.float32)
    src = np.asarray(src, dtype=np.int32)
    dst = np.asarray(dst, dtype=np.int32)

    consts, per_core, meta = _preprocess(y, ex, W1, b1, W2, b2, src, dst)
    nc = _build_program(meta)

    in_maps = []
    for c in range(N_CORES):
        in_maps.append({
            "featT": per_core["featT"][c],
            "Omat": per_core["O"][c],
            "cinv": per_core["cinv"][c],
            "W1f2": consts["W1f2"],
            "W2b": consts["W2b"],
            "b2": consts["b2"],
        })
    res = run_bass_kernel_spmd(nc, in_maps, list(range(N_CORES)))

    NPC = meta["NPC"]
    h = np.empty((meta["N"], 32), dtype=np.float32)
    for c in range(N_CORES):
        h[c * NPC : (c + 1) * NPC, :] = res.results[c]["hT"].T
    return h


# revision 25
# speedup vs baseline: 1.3840x; 1.3840x over previous
"""Trainium2 Bass kernel for GNN mean-aggregation message passing.

  m = relu(concat(y[src], ex) @ W1.T + b1)        per edge
  z = segment_mean(m, dst)                        per node (0 for isolated)
  h = relu(z @ W2.T + b2)                         per node

Strategy (8 NeuronCores, one SPMD program, dst-sharded):
  - Host shards edges by dst node range (N/8 nodes per core) and sorts each
    core's edges by (psum-window, dst). Per-window tile counts and per-tile
    one-hot column spans are unioned across cores so a single program fits
    all shards (padding slots carry zero features and zero one-hot weight).
  - The full per-edge feature matrix feat = [y[src]; ex; 1] is gathered on
    the HOST into a pair-packed edge-slot layout ([98, E_slots/2] bf16 per
    core: tile 2i in rows 0:49, tile 2i+1 in rows 49:98), so the device
    never does an indirect gather, and one PE matmul against the
    block-diagonal [[W1';0],[0;W1']] computes TWO 128-edge message tiles
    per LDWEIGHTS (bias folded via the ones row) -> relu -> m.
  - Scatter-sum on PE: s^T[48, win] += m[128e, 48].T @ O[128e, span], with
    O a 0/1 one-hot (exact in fp8). The mean is applied at window drain
    as an elementwise multiply with 1/deg -> bf16 z.
  - Node MLP: h^T[32, win] = relu(W2.T @ z^T + b2) streamed per window.
"""

import numpy as np
import ml_dtypes

N_CORES = 8
WIN = 512          # nodes per PSUM scatter window (1 bank, no span splits)
TILE_E = 128       # edges per scatter matmul (PE contraction dim)
SUPER = 8          # tiles (= SUPER//2 pairs) per relu batch

BF16 = ml_dtypes.bfloat16


def _preprocess(y, ex, W1, b1, W2, b2, src, dst):
    N, ND = y.shape
    E, ED = ex.shape
    D = ND + ED
    K1 = D + 1
    NPC = N // N_CORES
    NW = (NPC + WIN - 1) // WIN

    cnt = np.bincount(dst, minlength=N)
    inv_cnt = (1.0 / np.maximum(cnt, 1)).astype(np.float32)

    core_of = (dst // NPC).astype(np.int64)
    win_of = ((dst - core_of * NPC) // WIN).astype(np.int64)
    cw = core_of * NW + win_of
    key = cw * np.int64(N + 1) + dst
    order = np.argsort(key, kind="stable")

    dst_s = dst[order].astype(np.int64)
    ex_s = ex[order]
    src_s = src[order].astype(np.int64)
    ys = y[src_s]
    core_s = core_of[order]
    win_s = win_of[order]
    cw_s = cw[order]

    cw_cnt = np.bincount(cw_s, minlength=N_CORES * NW).reshape(N_CORES, NW)
    T_w = (cw_cnt.max(axis=0) + TILE_E - 1) // TILE_E          # [NW]
    T_w = ((T_w + 1) // 2) * 2                                 # even (pairs)
    win_block_base = np.concatenate([[0], np.cumsum(T_w)])
    B_tot = int(win_block_base[-1])
    E_slots = B_tot * TILE_E

    # rank of each edge within its (core, window) run
    cw_start = np.zeros(N_CORES * NW + 1, np.int64)
    cw_start[1:] = np.cumsum(cw_cnt.reshape(-1))
    rank = np.arange(E, dtype=np.int64) - cw_start[cw_s]
    slot = win_block_base[win_s] * TILE_E + rank
    tile_of = slot // TILE_E
    p_in_tile = slot % TILE_E

    # per-tile node span (relative to window start), unioned over cores
    rel = dst_s - core_s * NPC - win_s * WIN
    lo_t = np.full(B_tot, np.int64(1 << 60))
    hi_t = np.full(B_tot, np.int64(-1))
    np.minimum.at(lo_t, tile_of, rel)
    np.maximum.at(hi_t, tile_of, rel)
    empty = hi_t < 0
    lo_t[empty] = 0
    hi_t[empty] = 0
    span_t = hi_t - lo_t + 1
    col_off = np.concatenate([[0], np.cumsum(span_t)])
    C_tot = int(col_off[-1])
    o_col = col_off[tile_of] + (rel - lo_t[tile_of])

    # pair-packed feature layout: tile 2i -> rows 0:K1, tile 2i+1 -> K1:2*K1
    fcol = (tile_of // 2) * TILE_E + p_in_tile
    frow = (tile_of % 2) * K1
    featT = np.zeros((N_CORES, 2 * K1, E_slots // 2), BF16)
    O_a = np.zeros((N_CORES, TILE_E, C_tot), ml_dtypes.float8_e4m3)
    for c in range(N_CORES):
        m = core_s == c
        fc = fcol[m]
        fr = frow[m]
        ysb = ys[m].astype(BF16)
        exb = ex_s[m].astype(BF16)
        for half in range(2):
            hm = fr == half * K1
            base = half * K1
            featT[c, base : base + ND, fc[hm]] = ysb[hm]
            featT[c, base + ND : base + D, fc[hm]] = exb[hm]
            featT[c, base + D, fc[hm]] = 1.0
        O_a[c, p_in_tile[m], o_col[m]] = 1.0

    cinv = np.empty((N_CORES, D, NPC), BF16)
    for c in range(N_CORES):
        cinv[c] = np.broadcast_to(inv_cnt[c * NPC : (c + 1) * NPC], (D, NPC))

    meta = {
        "N": N, "E": E, "ND": ND, "ED": ED, "D": D, "NPC": NPC,
        "n_win": NW, "T_w": T_w, "win_block_base": win_block_base,
        "B_tot": B_tot, "E_slots": E_slots,
        "C_tot": C_tot, "lo_t": lo_t, "span_t": span_t, "col_off": col_off,
    }
    W1f = np.concatenate([W1.T, b1[None, :]], 0).astype(BF16)   # [K1, D]
    W1f2 = np.zeros((2 * K1, 2 * D), BF16)
    W1f2[:K1, :D] = W1f
    W1f2[K1:, D:] = W1f
    consts = dict(
        W1f2=np.ascontiguousarray(W1f2),
        W2b=np.ascontiguousarray(W2.T).astype(BF16),
        b2=np.ascontiguousarray(b2.reshape(-1, 1)).astype(np.float32),
    )
    per_core = dict(featT=featT, O=O_a, cinv=cinv)
    return consts, per_core, meta


def _split_excess_waits(nc, mybir):
    """This walrus build accepts at most 1 sync wait per instruction (0 on
    Drain). Move extras onto NOPs inserted just before, same engine."""
    for fn in nc.m.functions:
        for bb in fn.blocks:
            new_list = []
            for ins in bb.instructions:
                si = ins.sync_info
                limit = 0 if isinstance(ins, mybir.InstDrain) else 1
                if si is not None and si.on_wait and len(si.on_wait) > limit:
                    waits = list(si.on_wait)
                    keep, extra = waits[:limit], waits[limit:]
                    while extra:
                        chunk, extra = extra[:1], extra[1:]
                        nop = mybir.InstNoOp(
                            name=nc.get_next_instruction_name(), ins=[], outs=[])
                        nop.engine = ins.engine
                        nop.sync_info = mybir.SyncInfo(on_wait=chunk, on_update=[])
                        nc.register_instruction(nop)
                        new_list.append(nop)
                    si.on_wait = keep
                new_list.append(ins)
            bb.instructions[:] = new_list


def _build_program(meta):
    import concourse.bacc as bacc
    import concourse.mybir as mybir
    import concourse.tile as tile

    f32 = mybir.dt.float32
    bf16 = mybir.dt.bfloat16
    f8 = mybir.dt.float8e4
    Relu = mybir.ActivationFunctionType.Relu
    MULT = mybir.AluOpType.mult
    MAX = mybir.AluOpType.max

    N, ND, ED, D = meta["N"], meta["ND"], meta["ED"], meta["D"]
    K1 = D + 1
    NPC, NW = meta["NPC"], meta["n_win"]
    T_w, wbb = meta["T_w"], meta["win_block_base"]
    E_slots, C_tot = meta["E_slots"], meta["C_tot"]
    lo_t, span_t, col_off = meta["lo_t"], meta["span_t"], meta["col_off"]
    OD = 32

    nc = bacc.Bacc("TRN2")
    feat_ext = nc.dram_tensor("featT", [2 * K1, E_slots // 2], bf16,
                              kind="ExternalInput")
    O_ext = nc.dram_tensor("Omat", [TILE_E, C_tot], f8, kind="ExternalInput")
    cinv_ext = nc.dram_tensor("cinv", [D, NPC], bf16, kind="ExternalInput")
    w1f2_ext = nc.dram_tensor("W1f2", [2 * K1, 2 * D], bf16,
                              kind="ExternalInput")
    w2b_ext = nc.dram_tensor("W2b", [D, OD], bf16, kind="ExternalInput")
    b2_ext = nc.dram_tensor("b2", [OD, 1], f32, kind="ExternalInput")
    out_ext = nc.dram_tensor("hT", [OD, NPC], f32, kind="ExternalOutput")

    with tile.TileContext(nc) as tc:
        with (
            tc.tile_pool(name="const", bufs=1) as cpool,
            tc.tile_pool(name="io", bufs=3) as iopool,
            tc.tile_pool(name="aux", bufs=3) as apool,
            tc.tile_pool(name="msb", bufs=4) as mpool,
            tc.tile_pool(name="psA", bufs=3, space="PSUM") as psA,
            tc.tile_pool(name="psZ", bufs=3, space="PSUM") as psZ,
            tc.tile_pool(name="psH", bufs=2, space="PSUM") as psH,
        ):
            w1f2_sb = cpool.tile([2 * K1, 2 * D], bf16)
            nc.sync.dma_start(out=w1f2_sb[:], in_=w1f2_ext[:])
            w2b_sb = cpool.tile([D, OD], bf16)
            nc.sync.dma_start(out=w2b_sb[:], in_=w2b_ext[:])
            b2_sb = cpool.tile([OD, 1], f32)
            nc.sync.dma_start(out=b2_sb[:], in_=b2_ext[:])
            zl_bf = cpool.tile([1, 128], bf16)
            nc.any.memset(zl_bf[:], 0)
            zr_bf = cpool.tile([1, 512], bf16)
            nc.any.memset(zr_bf[:], 0)

            # Software-pipelined emission: scatters trail their msg supers by
            # TWO supers so the ACT/DVE relu (split in half across both
            # engines) has a full super period of slack before the scatter
            # needs m_sb. Window drains trail by a further two supers.
            from collections import deque
            state = {"scatters": deque(), "drain_due": None, "drain_next": None}

            def pop_pending():
                if state["drain_due"] is not None:
                    state["drain_due"]()
                state["drain_due"] = state["drain_next"]
                state["drain_next"] = None
                if len(state["scatters"]) > 1:
                    state["scatters"].popleft()()

            def flush_pending():
                while (state["scatters"] or state["drain_due"] is not None
                       or state["drain_next"] is not None):
                    if state["drain_due"] is not None:
                        state["drain_due"]()
                    state["drain_due"] = state["drain_next"]
                    state["drain_next"] = None
                    if state["scatters"]:
                        state["scatters"].popleft()()

            for w in range(NW):
                B = int(T_w[w])
                if B == 0:
                    continue
                wn = min(WIN, NPC - w * WIN)
                b0 = int(wbb[w])
                gn2 = (B // 2) * TILE_E          # pair-packed columns
                e20 = (b0 // 2) * TILE_E

                feat_t = iopool.tile([2 * K1, gn2], bf16, tag="feat")
                nc.gpsimd.dma_start(
                    out=feat_t[:], in_=feat_ext[:, e20 : e20 + gn2])
                csp = int(col_off[b0 + B] - col_off[b0])
                o_t = apool.tile([TILE_E, csp], f8, tag="omat")
                nc.scalar.dma_start(
                    out=o_t[:],
                    in_=O_ext[:, int(col_off[b0]) : int(col_off[b0]) + csp])
                cinv_t = apool.tile([D, WIN], bf16, tag="cinv")
                nc.sync.dma_start(
                    out=cinv_t[:, :wn], in_=cinv_ext[:, w * WIN : w * WIN + wn])

                psz = psZ.tile([128, WIN], f32, tag="psz")
                nc.tensor.matmul(
                    psz[:, :WIN], zl_bf[:], zr_bf[:, :WIN],
                    start=True, stop=True)

                n_super = (B + SUPER - 1) // SUPER
                for si in range(n_super):
                    s = si * SUPER
                    sb = min(SUPER, B - s)
                    npair = (sb + 1) // 2
                    ps_a = psA.tile([TILE_E, (SUPER // 2) * 2 * D], f32,
                                    tag="psa")
                    for pr in range(npair):
                        p0 = (s // 2) + pr
                        nc.tensor.matmul(
                            ps_a[:, pr * 2 * D : (pr + 1) * 2 * D],
                            feat_t[:, p0 * TILE_E : (p0 + 1) * TILE_E],
                            w1f2_sb[:], start=True, stop=True)
                    # m in fp8, each tile padded to 128 cols so the scatter
                    # LDWEIGHTS gets Fast Weight Load (4 fp8/cycle); the pad
                    # columns are garbage and only land in psz rows 48-127,
                    # which are never read.
                    m_sb = mpool.tile([TILE_E, SUPER * 128], f8, tag="m")
                    m_out = m_sb[:, : sb * 128].rearrange(
                        "p (b c) -> p b c", c=128)[:, :, :D]
                    m_in = ps_a[:, : sb * D].rearrange("p (b d) -> p b d", d=D)
                    if si % 2 == 0 or si == n_super - 1:
                        nc.scalar.activation(out=m_out, in_=m_in, func=Relu)
                    else:
                        nc.vector.tensor_scalar(
                            out=m_out, in0=m_in,
                            scalar1=0.0, scalar2=None, op0=MAX)
                    pop_pending()

                    def _scatter(w=w, wn=wn, b0=b0, s=s, sb=sb, si=si,
                                 n_super=n_super, m_sb=m_sb, psz=psz,
                                 o_t=o_t, cinv_t=cinv_t):
                        for t in range(sb):
                            bt = b0 + s + t
                            lo = int(lo_t[bt])
                            sp = int(span_t[bt])
                            off = int(col_off[bt] - col_off[b0])
                            last = si == n_super - 1 and t == sb - 1
                            nc.tensor.matmul(
                                psz[:, lo : lo + sp],
                                m_sb[:, t * 128 : (t + 1) * 128],
                                o_t[:, off : off + sp],
                                start=False, stop=last,
                                skip_group_check=True)
                        if last:
                            zt = mpool.tile([D, WIN], bf16, tag="zt")
                            nc.vector.tensor_tensor(
                                out=zt[:, :wn], in0=psz[:D, :wn],
                                in1=cinv_t[:, :wn], op=MULT)

                            def _drain(w=w, wn=wn, zt=zt):
                                ps_h = psH.tile([OD, WIN], f32, tag="psh")
                                nc.tensor.matmul(
                                    ps_h[:, :wn], w2b_sb[:], zt[:, :wn],
                                    start=True, stop=True)
                                h_sb = mpool.tile([OD, WIN], f32, tag="h")
                                nc.scalar.activation(
                                    out=h_sb[:, :wn], in_=ps_h[:, :wn],
                                    func=Relu, bias=b2_sb[:, 0:1])
                                nc.sync.dma_start(
                                    out=out_ext[:, w * WIN : w * WIN + wn],
                                    in_=h_sb[:, :wn])

                            state["drain_next"] = _drain

                    state["scatters"].append(_scatter)
            flush_pending()

    nc.compile()
    _split_excess_waits(nc, mybir)
    return nc


def kernel(y, ex, W1, b1, W2, b2, src, dst):
    from concourse.bass_utils import run_bass_kernel_spmd

    y = np.asarray(y, dtype=np.float32)
    ex = np.asarray(ex, dtype=np.float32)
    W1 = np.asarray(W1, dtype=np.float32)
    b1 = np.asarray(b1, dtype=np.float32)
    W2 = np.asarray(W2, dtype=np.float32)
    b2 = np.asarray(b2, dtype=np.float32)
    src = np.asarray(src, dtype=np.int32)
    dst = np.asarray(dst, dtype=np.int32)

    consts, per_core, meta = _preprocess(y, ex, W1, b1, W2, b2, src, dst)
    nc = _build_program(meta)

    in_maps = []
    for c in range(N_CORES):
        in_maps.append({
            "featT": per_core["featT"][c],
            "Omat": per_core["O"][c],
            "cinv": per_core["cinv"][c],
            "W1f2": consts["W1f2"],
            "W2b": consts["W2b"],
            "b2": consts["b2"],
        })
    res = run_bass_kernel_spmd(nc, in_maps, list(range(N_CORES)))

    NPC = meta["NPC"]
    h = np.empty((meta["N"], 32), dtype=np.float32)
    for c in range(N_CORES):
        h[c * NPC : (c + 1) * NPC, :] = res.results[c]["hT"].T
    return h
